# revision 21
# baseline (speedup 1.0000x reference)
"""GCN2 (16-layer) kernel for 8 Trainium2 NeuronCores — v2.

Structure (per core, feat-major dense phase):
  - Node->tile packing (host): balance per-(slab, tile) edge counts to
    minimize SpMM chunk padding; table quarters (25/25/24/24 tiles) are
    the AllGather pipeline chunks AND the int16 gather windows.
  - SpMM: dma_gather source rows from private per-quarter HBM tables
    (4 SWDGE queues), flipped PE matmul (stationary=msgs, moving=S) giving
    feat-major PSUM accumulated straight into mT (mT pre-initialized with
    0.1*x0 on ACT).
  - Dense: aW via PE with fused DVE add + accum_out sums; BN sumsq via ACT
    Square accum_out; 1-beta dropped (BN scale invariance); apply+residual+
    relu in 2 fused DVE passes.
  - Table update: per quarter: PE transpose -> ACT copy -> DMA -> AllGather
    (shared) -> HBM copy to private; next layer's slab-q gathers chain on
    copy_q so 3/4 of the collective hides under SpMM.
"""
import os
import time
import numpy as np
import ml_dtypes

import jax
import concourse.bacc as bacc
import concourse.mybir as mybir
import concourse.tile as tile
from concourse.ap import AP as _AP
from concourse.masks import make_identity

N = 100000
E = 800000
D = 128
D_OUT = 40
L = 16
ALPHA = 0.1
THETA = 0.5
BN_EPS = 1e-5

NC = 8
P = 128
SH = N // NC            # 12500
NT = (SH + P - 1) // P  # 98
SHP = NT * P            # 12544
NSLAB = 4
NTQ = [25, 25, 24, 24]
QT0 = [0, 25, 50, 74, 98]
CHUNK = 128
CALL_CHUNKS = int(os.environ.get("KCC", "16"))
SBATCH = 16
NQ = 4
MSG_BUFS = int(os.environ.get("KMB", "10"))
PAIR = int(os.environ.get("KPAIR", "0"))  # 512B pair-fetch descriptors
GCAP = NT * CHUNK * 2

BETAS = [float(np.log(THETA / l + 1.0)) for l in range(1, L + 1)]
AT = mybir.AluOpType
AF = mybir.ActivationFunctionType

_cache = {}


# ---------------------------------------------------------------- packing --
def _refine_tiles(dn, assign_t, cnt, cap, budget_s, rng):
    dn64 = dn.astype(np.int64)
    t_start = time.time()
    best_assign = assign_t.copy()
    best_over = np.maximum(cnt - cap, 0).sum()
    stall = 0
    while time.time() - t_start < budget_s:
        flat = np.maximum(cnt - cap, 0)
        total_over = flat.sum()
        if total_over < best_over:
            best_over = total_over
            best_assign = assign_t.copy()
        if total_over == 0:
            break
        over_cells = np.argwhere(flat > 0)
        t1, s1 = over_cells[rng.integers(len(over_cells))]
        t1, s1 = int(t1), int(s1)
        mem1 = np.flatnonzero(assign_t == t1)
        dmem = dn64[mem1]
        top = np.argsort(-(dmem[:, s1] * 100 + dmem.sum(axis=1)))
        moved = False
        for ui in top[:4]:
            u = int(mem1[ui])
            du = dn64[u]
            base1 = cnt[t1].astype(np.int64) - du
            cnt_v = cnt[assign_t].astype(np.int64)
            cap_v = cap[assign_t, 0][:, None] if cap.ndim == 2 else cap
            cap_1 = cap[t1] if cap.ndim == 2 else cap
            o1 = np.maximum(base1[None, :] + dn64 - cap_1, 0).sum(axis=1)
            o2 = np.maximum(cnt_v - dn64 + du[None, :] - cap_v, 0).sum(axis=1)
            cur = (np.maximum(cnt[t1] - cap_1, 0).sum()
                   + np.maximum(cnt_v - cap_v, 0).sum(axis=1))
            delta = o1 + o2 - cur
            delta[assign_t == t1] = 10**9
            v = int(np.argmin(delta))
            if int(delta[v]) < 0:
                t2 = int(assign_t[v])
                dv = dn64[v]
                cnt[t1] += (dv - du).astype(np.int32)
                cnt[t2] += (du - dv).astype(np.int32)
                assign_t[u] = t2
                assign_t[v] = t1
                moved = True
                break
        if moved:
            stall = 0
            continue
        stall += 1
        if stall >= 3:
            for _ in range(2):
                flat = np.maximum(cnt - cap, 0)
                oc = np.argwhere(flat > 0)
                if len(oc) == 0:
                    break
                t1k, s1k = oc[rng.integers(len(oc))]
                m1 = np.flatnonzero(assign_t == t1k)
                pick = m1[dn64[m1][:, s1k] > 0]
                if len(pick) == 0:
                    continue
                u = int(rng.choice(pick))
                capcol = cap[:, 0] if cap.ndim == 2 else cap
                slack = capcol - cnt[:, s1k]
                t2 = int(rng.choice(np.argsort(-slack)[:8]))
                if t2 == int(assign_t[u]):
                    continue
                m2 = np.flatnonzero(assign_t == t2)
                v = int(rng.choice(m2))
                tu, tv = int(assign_t[u]), int(assign_t[v])
                du, dv = dn64[u], dn64[v]
                cnt[tu] += (dv - du).astype(np.int32)
                cnt[tv] += (du - dv).astype(np.int32)
                assign_t[u], assign_t[v] = tv, tu
            stall = 0
    flat = np.maximum(cnt - cap, 0)
    if flat.sum() > best_over:
        assign_t[:] = best_assign
        cnt[:] = 0
        np.add.at(cnt, assign_t, dn.astype(np.int32))
    return assign_t, cnt


def _pack(edge_row, edge_col, seed=7, p1_budget=4.0, p2_budget=1.5):
    edge_row = np.asarray(edge_row, dtype=np.int64)
    edge_col = np.asarray(edge_col, dtype=np.int64)
    rng = np.random.default_rng(seed)

    odeg = np.zeros((N, NC), dtype=np.int32)
    np.add.at(odeg, (edge_col, edge_row // SH), 1)
    quarter = np.zeros(N, dtype=np.int32)
    for c in range(NC):
        dn = odeg[c * SH:(c + 1) * SH]
        sizes = np.array([q * P for q in NTQ])
        sizes[2] -= 22
        sizes[3] -= 22
        order = np.argsort(-dn.sum(axis=1), kind="stable")
        qcnt = np.zeros((4, NC), dtype=np.int64)
        qocc = np.zeros(4, dtype=np.int64)
        target = dn.sum(axis=0) / 4.0
        qa = np.zeros(SH, dtype=np.int32)
        for j in order:
            dv = dn[j]
            score = ((qcnt + dv - target[None, :]).clip(0).sum(axis=1)
                     + 0.001 * qocc)
            score = np.where(qocc >= sizes, np.inf, score)
            q = int(np.argmin(score))
            qa[j] = q
            qcnt[q] += dv
            qocc[q] += 1
        quarter[c * SH:(c + 1) * SH] = qa

    g = np.zeros((4, NC), dtype=np.int64)
    np.add.at(g, (quarter[edge_col], edge_row // SH), 1)
    odeg64 = odeg.astype(np.int64)
    t_start = time.time()
    fails = 0
    while time.time() - t_start < p1_budget and fails < 20:
        over = np.maximum(g - GCAP, 0)
        if over.sum() == 0:
            break
        qq, cc = np.unravel_index(np.argmax(over), over.shape)
        qq, cc = int(qq), int(cc)
        improved = False
        for c in rng.permutation(NC):
            ids = np.arange(c * SH, (c + 1) * SH)
            qa = quarter[ids]
            in_q = ids[qa == qq]
            if len(in_q) == 0:
                continue
            u = int(in_q[np.argmax(odeg64[in_q, cc])])
            du = odeg64[u]
            others = ids[qa != qq]
            dv_all = odeg64[others]
            qv = quarter[others]
            gq = g[qq][None, :] + dv_all - du[None, :]
            gv = g[qv] + du[None, :] - dv_all
            o_new = (np.maximum(gq - GCAP, 0).sum(axis=1)
                     + np.maximum(gv - GCAP, 0).sum(axis=1))
            o_cur = (np.maximum(g[qq] - GCAP, 0).sum()
                     + np.maximum(g[qv] - GCAP, 0).sum(axis=1))
            delta = o_new - o_cur
            vi = int(np.argmin(delta))
            if int(delta[vi]) < 0:
                v = int(others[vi])
                q2 = int(quarter[v])
                g[qq] += odeg64[v] - du
                g[q2] += du - odeg64[v]
                quarter[u] = q2
                quarter[v] = qq
                improved = True
                break
        if not improved:
            fails += 1
            if fails % 5 == 0:
                # kick: force-swap a random high-degree node out of the
                # worst cell to escape the plateau
                c = int(rng.integers(NC))
                ids = np.arange(c * SH, (c + 1) * SH)
                in_q = ids[quarter[ids] == qq]
                others = ids[quarter[ids] != qq]
                if len(in_q) and len(others):
                    u = int(rng.choice(in_q))
                    v = int(rng.choice(others))
                    q2 = int(quarter[v])
                    g[qq] += odeg64[v] - odeg64[u]
                    g[q2] += odeg64[u] - odeg64[v]
                    quarter[u] = q2
                    quarter[v] = qq
            if fails >= 60:
                break

    d = np.zeros((N, NSLAB), dtype=np.int32)
    np.add.at(d, (edge_row, quarter[edge_col]), 1)
    pos = np.zeros(N, dtype=np.int64)
    Kmats = np.zeros((NC, NSLAB, NT), dtype=np.int64)
    for c in range(NC):
        for q in range(4):
            ids = np.flatnonzero(quarter[c * SH:(c + 1) * SH] == q) + c * SH
            dn = d[ids]
            ntq = NTQ[q]
            cnt = np.zeros((ntq, NSLAB), dtype=np.int32)
            occ = np.zeros(ntq, dtype=np.int32)
            at = np.zeros(len(ids), dtype=np.int32)
            order = np.argsort(-(dn.max(axis=1) * 1000 + dn.sum(axis=1)),
                               kind="stable")
            # two designated overflow tiles (K=3) per quarter, aligned
            # across cores by the post-pack K-pattern sort; the rest are
            # held strictly to K=2
            cap2 = np.full((ntq, 1), 2 * CHUNK, dtype=np.int64)
            cap2[:2] = 3 * CHUNK
            for j in order:
                dv = dn[j]
                over = np.maximum(cnt + dv - (cap2 - 2), 0).sum(axis=1)
                score = over * 1e6 + (cnt + dv).sum(axis=1)
                score = np.where(occ >= P, np.inf, score)
                t = int(np.argmin(score))
                at[j] = t
                cnt[t] += dv
                occ[t] += 1
            at, cnt = _refine_tiles(dn, at, cnt, cap2, p2_budget, rng)
            K_c = np.ceil(cnt / CHUNK).astype(np.int64)
            tile_order = sorted(range(ntq), key=lambda t: tuple(-K_c[t]))
            inv = np.zeros(ntq, dtype=np.int64)
            for newt, oldt in enumerate(tile_order):
                inv[oldt] = newt
            at2 = inv[at]
            Kmats[c, :, QT0[q]:QT0[q] + ntq] = K_c[tile_order].T
            idx_sorted = np.argsort(at2, kind="stable")
            fill = np.zeros(ntq, dtype=np.int64)
            for jj in idx_sorted:
                t = at2[jj]
                pos[ids[jj]] = (QT0[q] + t) * P + fill[t]
                fill[t] += 1

    K_st = Kmats.max(axis=0)
    K_st[:, 0] = np.maximum(K_st[:, 0], 1)
    return pos, quarter, K_st


# ------------------------------------------------------------------- prep --
def _prep(edge_row, edge_col, edge_weight):
    edge_row = np.asarray(edge_row).astype(np.int64)
    edge_col = np.asarray(edge_col).astype(np.int64)
    w = np.asarray(edge_weight).astype(np.float64)

    pos, quarter, K_st = _pack(edge_row, edge_col)

    core = edge_row // SH
    posr = pos[edge_row]
    t_arr = posr // P
    m_arr = posr % P
    s_arr = quarter[edge_col].astype(np.int64)
    # table row within quarter-table: rank*(128*ntq) + m_src*ntq + (t-qt0)
    posc = pos[edge_col]
    corec = edge_col // SH
    t_src = posc // P
    m_src = posc % P
    ntq_arr = np.asarray(NTQ)[s_arr]
    qt0_arr = np.asarray(QT0)[:4][s_arr]
    idx_arr = corec * (P * ntq_arr) + m_src * ntq_arr + (t_src - qt0_arr)
    assert (idx_arr >= 0).all() and (idx_arr < 32768).all()

    # schedule from K_st
    seg_start = np.zeros((NSLAB, NT), dtype=np.int64)
    c = 0
    call_meta = []
    slab_bounds = []
    for s in range(NSLAB):
        s0 = c
        for t in range(NT):
            seg_start[s, t] = c
            c += K_st[s, t]
        slab_bounds.append((s0, c))
    C_total = int(c)
    total_slots = C_total * CHUNK

    callmap = np.zeros(total_slots, dtype=np.int64)
    colbase = 0
    for s in range(NSLAB):
        s0, s1 = slab_bounds[s]
        off = s0
        while off < s1:
            nch = min(CALL_CHUNKS, s1 - off)
            nidx = nch * CHUNK
            gidx = np.arange(nidx)
            callmap[off * CHUNK + gidx] = (colbase + gidx // 16) * 16 + (gidx % 16)
            call_meta.append((s, off, nch, colbase))
            colbase += nidx // 16
            off += nch
    idx_cols = colbase

    idx16 = np.zeros((NC, 16, idx_cols), dtype=np.int16)
    S = np.zeros((NC, P, C_total, P), dtype=ml_dtypes.bfloat16)
    order = np.lexsort((idx_arr, t_arr, s_arr, core))
    eo_core = core[order]
    eo_s = s_arr[order]
    eo_t = t_arr[order]
    eo_m = m_arr[order]
    eo_idx = idx_arr[order]
    eo_w = (w[order] * (1.0 - ALPHA)).astype(np.float32)

    for ci in range(NC):
        msk = eo_core == ci
        es, et = eo_s[msk], eo_t[msk]
        seg_id = es * NT + et
        n = len(seg_id)
        posn = np.zeros(n, dtype=np.int64)
        if n:
            change = np.concatenate([[True], seg_id[1:] != seg_id[:-1]])
            starts = np.flatnonzero(change)
            run = np.arange(n)
            posn = run - np.repeat(run[starts],
                                   np.diff(np.concatenate([starts, [n]])))
        slot = seg_start[es, et] * CHUNK + posn
        gc = callmap[slot]
        idx16[ci, gc % 16, gc // 16] = eo_idx[msk].astype(np.int16)
        S[ci, slot % CHUNK, slot // CHUNK, eo_m[msk]] = eo_w[msk]
    idx16 = np.tile(idx16, (1, 8, 1))

    sched = {
        "C_total": C_total,
        "idx_cols": int(idx_cols),
        "call_meta": call_meta,
        "seg_start": seg_start,
        "K_st": K_st,
    }
    return sched, idx16, S, pos


# ------------------------------------------------------------------ build --
def _build_nc(sched, n_layers=L, parts=15):
    nc = bacc.Bacc("TRN2", num_swdge_queues=NQ)
    bf16 = mybir.dt.bfloat16
    f32 = mybir.dt.float32
    C_total = sched["C_total"]
    idx_cols = sched["idx_cols"]
    call_meta = sched["call_meta"]
    seg_start = sched["seg_start"]
    K_st = sched["K_st"]

    xT_in = nc.dram_tensor("xT", [P, SHP], f32, kind="ExternalInput")
    idx_in = nc.dram_tensor("idx16", [P, idx_cols], mybir.dt.int16,
                            kind="ExternalInput")
    S_in = nc.dram_tensor("S", [P, C_total, P], bf16, kind="ExternalInput")
    Win_in = nc.dram_tensor("W_in", [P, P], f32, kind="ExternalInput")
    binT_in = nc.dram_tensor("b_inT", [P, 1], f32, kind="ExternalInput")
    Wst_in = nc.dram_tensor("W_stat", [P, L * P], bf16, kind="ExternalInput")
    gamT_in = nc.dram_tensor("gammaT", [P, L], f32, kind="ExternalInput")
    betT_in = nc.dram_tensor("betaT", [P, L], f32, kind="ExternalInput")
    Wout_in = nc.dram_tensor("W_outT", [P, D_OUT], bf16, kind="ExternalInput")
    bout_in = nc.dram_tensor("b_outR", [P, D_OUT], f32, kind="ExternalInput")
    out_ext = nc.dram_tensor("out", [P, NT, D_OUT], f32, kind="ExternalOutput")

    ag_ins = [[nc.dram_tensor(f"ag_in{i}_{q}", [P * NTQ[q], P], bf16)
               for q in range(4)] for i in range(2)]
    tab_sh = [[nc.dram_tensor(f"tabs{i}_{q}", [NC * P * NTQ[q], P], bf16,
                              addr_space="Shared")
               for q in range(4)] for i in range(2)]
    tab_pv = [[nc.dram_tensor(f"tabp{i}_{q}", [NC * P * NTQ[q] + PAIR, P],
                              bf16)
               for q in range(4)] for i in range(2)]
    ar_ins = [nc.dram_tensor(f"ar_in{i}", [P, 2], f32) for i in range(2)]
    ar_outs = [nc.dram_tensor(f"ar_out{i}", [P, 2], f32, addr_space="Shared")
               for i in range(2)]
    rg = [list(range(NC))]

    NBLK = (SHP + 511) // 512  # 25 aW blocks

    with tile.TileContext(nc) as tc:
        with tc.tile_pool(name="persist", bufs=1) as pp, \
             tc.tile_pool(name="msgs", bufs=MSG_BUFS) as mp, \
             tc.tile_pool(name="spool", bufs=2) as sp, \
             tc.tile_pool(name="stg", bufs=2) as gp, \
             tc.tile_pool(name="sc", bufs=2) as scp, \
             tc.tile_pool(name="ps_spmm", bufs=3, space="PSUM") as ps_spmm, \
             tc.tile_pool(name="ps_tr", bufs=2, space="PSUM") as ps_tr, \
             tc.tile_pool(name="ps_w", bufs=2, space="PSUM") as ps_w:

            iden = pp.tile([P, P], bf16)
            make_identity(nc, iden[:])
            idxt = pp.tile([P, idx_cols], mybir.dt.int16)
            nc.sync.dma_start(idxt[:], idx_in[:])
            x0s = pp.tile([P, SHP], bf16)     # 0.1 * x0
            hT = pp.tile([P, SHP], bf16)
            mT = pp.tile([P, SHP], bf16)
            WinT = pp.tile([P, P], f32)
            binT = pp.tile([P, 1], f32)
            Wst = pp.tile([P, L * P], bf16)
            gamT = pp.tile([P, L], f32)
            betT = pp.tile([P, L], f32)
            WoutT = pp.tile([P, D_OUT], bf16)
            boutT = pp.tile([P, D_OUT], f32)
            stats = pp.tile([P, 2], f32)
            sumc = pp.tile([P, NBLK], f32)
            sqc = pp.tile([P, NBLK], f32)
            bnv = pp.tile([P, 6], f32)
            nc.sync.dma_start(WinT[:], Win_in[:])
            nc.sync.dma_start(binT[:], binT_in[:])
            nc.sync.dma_start(Wst[:], Wst_in[:])
            nc.sync.dma_start(gamT[:], gamT_in[:])
            nc.sync.dma_start(betT[:], betT_in[:])
            nc.sync.dma_start(WoutT[:], Wout_in[:])
            nc.sync.dma_start(boutT[:], bout_in[:])

            # PE warmup probe
            wps = ps_tr.tile([P, P], bf16, space="PSUM", tag="tr")
            nc.tensor.transpose(wps[:], iden[:], iden[:])

            if PAIR:
                zrow = pp.tile([1, P], bf16)
                nc.vector.memset(zrow[:], 0.0)
                for i in range(2):
                    for q in range(4):
                        nrows = NC * P * NTQ[q]
                        nc.sync.dma_start(
                            tab_pv[i][q][nrows:nrows + 1, :], zrow[:])

            def emit_quarter(li, q):
                pg = li % 2
                ntq = NTQ[q]
                stage = gp.tile([P, NTQ[0], P], bf16, tag="agstage")
                for tq in range(ntq):
                    t = QT0[q] + tq
                    pst = ps_tr.tile([P, P], bf16, space="PSUM", tag="tr")
                    nc.tensor.transpose(pst[:], hT[:, t * P:(t + 1) * P],
                                        iden[:])
                    nc.scalar.copy(stage[:, tq, :], pst[:])
                nc.sync.dma_start(
                    ag_ins[pg][q].ap().rearrange("(m t) f -> m t f", t=ntq),
                    stage[:, :ntq, :])
                nc.gpsimd.collective_compute(
                    "AllGather", AT.bypass,
                    ins=[ag_ins[pg][q].ap().opt()],
                    outs=[tab_sh[pg][q].ap().opt()],
                    replica_groups=rg)
                nrows = NC * P * ntq
                nc.sync.dma_start(tab_pv[pg][q][0:nrows, :],
                                  tab_sh[pg][q][:])

            def emit_table_update(li):
                for q in range(4):
                    emit_quarter(li, q)

            # ---- x0 stage: hT = relu((x @ W_in)^T + b), x0s = 0.1*hT ----
            # interleaved with the initial table update per quarter
            for q in range(4):
                for tq in range(NTQ[q]):
                    t = QT0[q] + tq
                    xt = scp.tile([P, P], f32, tag="xtile")
                    nc.sync.dma_start(xt[:], xT_in[:, t * P:(t + 1) * P])
                    ps = ps_w.tile([P, 512], f32, space="PSUM", tag="w")
                    nc.tensor.matmul(ps[:, :P], lhsT=WinT[:], rhs=xt[:],
                                     start=True, stop=True)
                    sl = slice(t * P, (t + 1) * P)
                    nc.vector.tensor_scalar(
                        out=hT[:, sl], in0=ps[:, :P],
                        scalar1=binT[:, :1], scalar2=0.0,
                        op0=AT.add, op1=AT.max)
                    nc.scalar.mul(x0s[:, sl], hT[:, sl], ALPHA)
                emit_quarter(0, q)

            seg_of = {}
            for s in range(NSLAB):
                for t in range(NT):
                    c0 = int(seg_start[s, t])
                    k = int(K_st[s, t])
                    for j in range(k):
                        seg_of[c0 + j] = (s, t, j == 0, j == k - 1)
            first_slab = {}
            last_slab = {}
            for t in range(NT):
                pres = [s for s in range(NSLAB) if K_st[s, t] > 0]
                first_slab[t] = pres[0]
                last_slab[t] = pres[-1]

            for li in range(n_layers):
                pg = li % 2

                # --- SpMM (slab-major) ---
                sbatch_bounds = []
                b0 = 0
                first = True
                while b0 < C_total:
                    nn = min(SBATCH // 2 if first else SBATCH, C_total - b0)
                    first = False
                    sbatch_bounds.append((b0, nn))
                    b0 += nn
                sb_iter = iter(sbatch_bounds)
                s_tiles = []
                cur_end = 0

                def s_ap_for(ch):
                    nonlocal cur_end
                    while ch >= cur_end:
                        c0, nn = next(sb_iter)
                        st = sp.tile([P, SBATCH, P], bf16, tag="S")
                        nc.sync.dma_start(st[:, :nn, :],
                                          S_in[:, c0:c0 + nn, :])
                        pa = ps_tr.tile([P, P], bf16, space="PSUM", tag="tr")
                        nc.tensor.transpose(pa[:2, :], st[:, 0, 0:2], iden[:])
                        s_tiles.append((c0, nn, st))
                        cur_end = c0 + nn
                    for c0, nn, st in reversed(s_tiles):
                        if c0 <= ch < c0 + nn:
                            return st[:, ch - c0, :]
                    raise AssertionError

                psums = {}
                ew = 2 * P if PAIR else P
                for ci, (s, c0_call, nch, col0) in enumerate(call_meta):
                    nidx = nch * CHUNK
                    msgs = mp.tile([P, CALL_CHUNKS, ew], bf16, tag="msgs")
                    if PAIR:
                        nrows = NC * P * NTQ[s]
                        src = _AP(tab_pv[pg][s], 0, [[P, nrows], [1, 2 * P]])
                        nc.gpsimd.dma_gather(
                            msgs[:, :nch, :], src,
                            idxt[:, col0:col0 + nidx // 16],
                            nidx, nidx, 2 * P, elem_step=P,
                            single_packet=False, queue_num=ci % NQ)
                    else:
                        nc.gpsimd.dma_gather(
                            msgs[:, :nch, :], tab_pv[pg][s][:],
                            idxt[:, col0:col0 + nidx // 16],
                            nidx, nidx, P, single_packet=False,
                            queue_num=ci % NQ)
                    pa = ps_tr.tile([P, P], bf16, space="PSUM", tag="tr")
                    nc.tensor.transpose(pa[:2, :], msgs[:, 0, 0:2], iden[:])
                    for j in range(nch):
                        ch = c0_call + j
                        sap = s_ap_for(ch)
                        ss, tt, segfirst, seglast = seg_of[ch]
                        if segfirst:
                            psums[tt] = ps_spmm.tile(
                                [P, P], f32, space="PSUM", tag="spmm",
                                name="pspmm")
                        # flipped: psum[feat, dest] += msgs^T @ S
                        nc.tensor.matmul(psums[tt][:], lhsT=msgs[:, j, 0:P],
                                         rhs=sap,
                                         start=segfirst, stop=seglast)
                        if seglast:
                            sl = slice(tt * P, (tt + 1) * P)
                            if ss == first_slab[tt]:
                                # first write: mT = psum + 0.1*x0
                                nc.vector.scalar_tensor_tensor(
                                    out=mT[:, sl], in0=psums[tt][:],
                                    scalar=0.0, in1=x0s[:, sl],
                                    op0=AT.add, op1=AT.add)
                            else:
                                nc.vector.tensor_tensor(
                                    out=mT[:, sl], in0=mT[:, sl],
                                    in1=psums[tt][:], op=AT.add)
                            del psums[tt]

                if parts < 2:
                    continue
                # --- aW + sums:  mT += (m @ W)^T  (W_stat = b/(1-b) W) ---
                WL = Wst[:, li * P:(li + 1) * P]
                for bi in range(NBLK):
                    t0 = bi * 512
                    nn = min(512, SHP - t0)
                    psw = ps_w.tile([P, 512], f32, space="PSUM", tag="w")
                    nc.tensor.matmul(psw[:, :nn], lhsT=WL, rhs=mT[:, t0:t0 + nn],
                                     start=True, stop=True)
                    nc.vector.scalar_tensor_tensor(
                        out=mT[:, t0:t0 + nn], in0=psw[:, :nn], scalar=0.0,
                        in1=mT[:, t0:t0 + nn], op0=AT.add, op1=AT.add,
                        accum_out=sumc[:, bi:bi + 1])

                if parts < 4:
                    continue
                # --- BN stats: sumsq on ACT, then AllReduce ---
                for bi in range(NBLK):
                    t0 = bi * 512
                    nn = min(512, SHP - t0)
                    junk = scp.tile([P, 512], bf16, tag="sqjunk")
                    nc.scalar.activation(junk[:, :nn], mT[:, t0:t0 + nn],
                                         AF.Square,
                                         accum_out=sqc[:, bi:bi + 1])
                nc.vector.tensor_reduce(out=stats[:, 0:1], in_=sumc[:],
                                        axis=mybir.AxisListType.X, op=AT.add)
                nc.vector.tensor_reduce(out=stats[:, 1:2], in_=sqc[:],
                                        axis=mybir.AxisListType.X, op=AT.add)
                nc.sync.dma_start(ar_ins[pg][:], stats[:])
                nc.gpsimd.collective_compute(
                    "AllReduce", AT.add,
                    ins=[ar_ins[pg].ap().opt()], outs=[ar_outs[pg].ap().opt()],
                    replica_groups=rg)
                arr = scp.tile([P, 2], f32, tag="arres")
                nc.sync.dma_start(arr[:], ar_outs[pg][:])
                if parts < 8:
                    continue
                nc.vector.tensor_scalar_mul(bnv[:, 0:2], arr[:], float(1.0 / N))
                nc.vector.tensor_tensor(out=bnv[:, 2:3], in0=bnv[:, 0:1],
                                        in1=bnv[:, 0:1], op=AT.mult)
                nc.vector.tensor_tensor(out=bnv[:, 2:3], in0=bnv[:, 1:2],
                                        in1=bnv[:, 2:3], op=AT.subtract)
                nc.vector.tensor_scalar_add(bnv[:, 2:3], bnv[:, 2:3], BN_EPS)
                nc.scalar.sqrt(bnv[:, 3:4], bnv[:, 2:3])
                nc.vector.reciprocal(bnv[:, 3:4], bnv[:, 3:4])
                nc.vector.tensor_tensor(out=bnv[:, 4:5], in0=bnv[:, 3:4],
                                        in1=gamT[:, li:li + 1], op=AT.mult)
                nc.vector.tensor_tensor(out=bnv[:, 5:6], in0=bnv[:, 0:1],
                                        in1=bnv[:, 4:5], op=AT.mult)
                nc.vector.tensor_tensor(out=bnv[:, 5:6],
                                        in0=betT[:, li:li + 1],
                                        in1=bnv[:, 5:6], op=AT.subtract)

                # --- apply + residual + relu:
                #     hT = max((mT*s1 + hT) + s2, 0) ---
                nc.vector.scalar_tensor_tensor(
                    out=hT[:], in0=mT[:], scalar=bnv[:, 4:5], in1=hT[:],
                    op0=AT.mult, op1=AT.add)
                nc.vector.tensor_scalar(
                    out=hT[:], in0=hT[:], scalar1=bnv[:, 5:6], scalar2=0.0,
                    op0=AT.add, op1=AT.max)

                if li < n_layers - 1:
                    emit_table_update(li + 1)

            # ---- output ----
            ostage = pp.tile([P, NT, D_OUT], f32)
            for t in range(NT):
                pso = ps_w.tile([P, 512], f32, space="PSUM", tag="w")
                nc.tensor.matmul(pso[:, :D_OUT],
                                 lhsT=hT[:, t * P:(t + 1) * P],
                                 rhs=WoutT[:], start=True, stop=True)
                nc.vector.tensor_tensor(out=ostage[:, t, :],
                                        in0=pso[:, :D_OUT], in1=boutT[:],
                                        op=AT.add)
            nc.sync.dma_start(out_ext[:], ostage[:])
    nc.compile()
    return nc


# ------------------------------------------------------------------ host --
def _make_inputs(inputs):
    W_in = np.asarray(inputs["W_in"], dtype=np.float32)
    b_in = np.asarray(inputs["b_in"], dtype=np.float32)
    conv_W = np.asarray(inputs["conv_W"], dtype=np.float32)
    bn_gamma = np.asarray(inputs["bn_gamma"], dtype=np.float32)
    bn_beta = np.asarray(inputs["bn_beta"], dtype=np.float32)
    W_out = np.asarray(inputs["W_out"], dtype=np.float32)
    b_out = np.asarray(inputs["b_out"], dtype=np.float32)

    W_stat = np.stack([conv_W[l] * (BETAS[l] / (1.0 - BETAS[l]))
                       for l in range(L)])
    W_stat = np.ascontiguousarray(
        W_stat.transpose(1, 0, 2).reshape(P, L * P)).astype(ml_dtypes.bfloat16)
    return {
        "W_in": W_in,
        "b_inT": np.ascontiguousarray(b_in[:, None]),
        "W_stat": W_stat,
        "gammaT": np.ascontiguousarray(bn_gamma.T),
        "betaT": np.ascontiguousarray(bn_beta.T),
        "W_outT": np.ascontiguousarray(W_out).astype(ml_dtypes.bfloat16),
        "b_outR": np.ascontiguousarray(np.tile(b_out[None, :], (P, 1))),
    }


def kernel(**inputs):
    if "runner" not in _cache:
        sched, idx16, S, pos = _prep(inputs["edge_row"], inputs["edge_col"],
                                     inputs["edge_weight"])
        nc = _build_nc(sched)
        r = _SpmdRunner(nc, NC)
        _cache["runner"] = (r, idx16, S, pos)
    r, idx16, S, pos = _cache["runner"]
    shared = _make_inputs(inputs)
    x = np.asarray(inputs["x"], dtype=np.float32)
    in_maps = []
    for c in range(NC):
        xs = np.zeros((SHP, P), dtype=np.float32)
        xs[pos[c * SH:(c + 1) * SH]] = x[c * SH:(c + 1) * SH]
        m = dict(shared)
        m["xT"] = np.ascontiguousarray(xs.T)
        m["idx16"] = idx16[c]
        m["S"] = np.ascontiguousarray(S[c])
        in_maps.append(m)
    dev_in = r.stage_inputs(in_maps)
    outs = r.run(dev_in)
    res = r.results(outs)
    full = np.zeros((N, D_OUT), dtype=np.float32)
    for c in range(NC):
        o = res[c]["out"].transpose(1, 0, 2).reshape(SHP, D_OUT)
        full[c * SH:(c + 1) * SH] = o[pos[c * SH:(c + 1) * SH]]
    return full


class _SpmdRunner:
    """Jit-once SPMD execution of a Bass module via PJRT/axon."""

    def __init__(self, nc, n_cores):
        from jax.sharding import Mesh, PartitionSpec
        from jax.experimental.shard_map import shard_map
        from concourse.bass2jax import (_bass_exec_p, install_neuronx_cc_hook,
                                        partition_id_tensor)
        install_neuronx_cc_hook()
        self.nc = nc
        self.n_cores = n_cores
        self.PartitionSpec = PartitionSpec
        self.shard_map = shard_map

        in_names, out_names, out_avals, zero_outs = [], [], [], []
        pname = nc.partition_id_tensor.name if nc.partition_id_tensor else None
        for alloc in nc.m.functions[0].allocations:
            if not isinstance(alloc, mybir.MemoryLocationSet):
                continue
            name = alloc.memorylocations[0].name
            if alloc.kind == "ExternalInput":
                if name != pname:
                    in_names.append(name)
            elif alloc.kind == "ExternalOutput":
                shape = tuple(alloc.tensor_shape)
                dtype = mybir.dt.np(alloc.dtype)
                out_names.append(name)
                out_avals.append(jax.core.ShapedArray(shape, dtype))
                zero_outs.append(np.zeros(shape, dtype))
        self.in_names, self.out_names = in_names, out_names
        self.out_avals, self.zero_outs = out_avals, zero_outs
        n_params, n_outs = len(in_names), len(out_names)
        self.n_params = n_params
        all_in = list(in_names) + list(out_names)
        if pname is not None:
            all_in.append(pname)
        donate = tuple(range(n_params, n_params + n_outs))

        def _body(*args):
            operands = list(args)
            if pname is not None:
                operands.append(partition_id_tensor())
            return tuple(_bass_exec_p.bind(
                *operands, out_avals=tuple(out_avals),
                in_names=tuple(all_in), out_names=tuple(out_names),
                lowering_input_output_aliases=(),
                sim_require_finite=True, sim_require_nnan=True, nc=nc))

        devices = jax.devices()[:n_cores]
        self.mesh = Mesh(np.asarray(devices), ("core",))
        self.fn = jax.jit(
            shard_map(_body, mesh=self.mesh,
                      in_specs=(PartitionSpec("core"),) * (n_params + n_outs),
                      out_specs=(PartitionSpec("core"),) * n_outs,
                      check_rep=False),
            donate_argnums=donate, keep_unused=True)

    def _ident(self, n):
        key = ("ident", n)
        if not hasattr(self, "_idents"):
            self._idents = {}
        if key not in self._idents:
            PS = self.PartitionSpec
            self._idents[key] = jax.jit(self.shard_map(
                lambda *a: tuple(a), mesh=self.mesh,
                in_specs=(PS("core"),) * n, out_specs=(PS("core"),) * n,
                check_rep=False))
        return self._idents[key]

    def stage_inputs(self, in_maps):
        per_core = [[np.asarray(m[n]) for n in self.in_names] for m in in_maps]
        concat = [np.concatenate([per_core[c][i] for c in range(self.n_cores)],
                                 axis=0) for i in range(self.n_params)]
        out = self._ident(len(concat))(*concat)
        jax.block_until_ready(out)
        return list(out)

    def _zero_args(self):
        zeros = [np.zeros((self.n_cores * z.shape[0], *z.shape[1:]), z.dtype)
                 for z in self.zero_outs]
        if not zeros:
            return []
        out = self._ident(len(zeros))(*zeros)
        jax.block_until_ready(out)
        return list(out)

    def run(self, dev_in):
        outs = self.fn(*dev_in, *self._zero_args())
        jax.block_until_ready(outs)
        return outs

    def results(self, outs):
        return [{name: np.asarray(outs[i]).reshape(
                    self.n_cores, *self.out_avals[i].shape)[c]
                 for i, name in enumerate(self.out_names)}
                for c in range(self.n_cores)]

    def time_runs(self, dev_in, iters=5):
        ts = []
        for _ in range(iters):
            za = self._zero_args()
            t0 = time.perf_counter()
            outs = self.fn(*dev_in, *za)
            jax.block_until_ready(outs)
            ts.append(time.perf_counter() - t0)
        return min(ts), ts
